# revision 10
# baseline (speedup 1.0000x reference)
"""Trainium2 Bass kernel for nn_ClustGeoNodeEncoder (segment_reduce).

Strategy (data-parallel over the cluster axis, per the sharding hint):
  - Host packs the voxel table as [N+1, 8] f32 rows: x, y, z, value,
    onehot(sem==1..4); row N is all-zeros and is the target of padded
    cluster slots.  (count of sem==0 is recovered as n - sum(oh1..4).)
  - Clusters are sorted by length and dealt round-robin to the 8 cores so
    every core compiles the same program (SPMD): 32 tiles x 128 clusters
    per core, tile t padded to Lb[t] = max length in its global rank range.
  - On device, each tile is gathered row-by-row with indirect DMA (one
    instruction gathers one 32B table row for each of the 128 clusters in
    the tile).  Padded slots fetch the zero row, so no masking is needed
    in the moment sums.
  - Pass A (per tile): raw sums / second moments / value stats / semantic
    counts via fused multiply-accumulate (scalar_tensor_tensor accum_out)
    and strided tensor_reduce; centered coordinates are retained in SBUF.
  - Batched per-cluster math on [128, 32] tiles: closed-form symmetric 3x3
    eigenvalues (trig method via Arctan/Sin on the scalar engine),
    principal eigenvector via the spectral projector (A - w0)(A - w1),
    B = A / w2, dirwt = 1 - w1/w2, mode via argmax scan.
  - Pass B (per tile): orientation statistic sc = sum(t * |xc_perp|) from
    the retained centered coords; padded slots contribute a closed-form
    correction term.  Sign-flip + dirwt scaling, then 19 output planes are
    DMA'd out and decoded on the host.
"""

import sys

for _p in ("/opt/trn_rl_repo",):
    if _p not in sys.path:
        sys.path.insert(0, _p)

import numpy as np

N = 2_000_000
C = 32768
L = 256
N_CORES = 8
P = 128
NT = C // (P * N_CORES)  # 32 tiles per core
f32 = np.float32

_PI = float(np.pi)


def _host_prep(data, clust_idx, clust_len):
    data = np.asarray(data, dtype=f32)
    clust_idx = np.asarray(clust_idx).astype(np.int32)
    lens = np.asarray(clust_len).astype(np.int64)

    table = np.zeros((N + 1, 8), dtype=f32)
    table[:N, 0:3] = data[:, 0:3]
    table[:N, 3] = data[:, 4]
    sem = data[:, 5].astype(np.int32)
    for k in range(1, 5):
        table[:N, 3 + k] = (sem == k)

    order = np.argsort(lens, kind="stable")  # ascending length
    # global rank r: tile t = r // (P * N_CORES); slot s = r % (P * N_CORES)
    # core = s % N_CORES ; partition = s // N_CORES
    Lb = np.zeros(NT, dtype=np.int64)
    for t in range(NT):
        Lb[t] = lens[order[t * P * N_CORES:(t + 1) * P * N_CORES]].max()
    S = int(Lb.sum())

    # padded index matrix [C, L] with invalid slots -> N (zero row)
    ar = np.arange(L)[None, :]
    idx_pad = np.where(ar < lens[:, None], clust_idx, N).astype(np.int32)

    idx_blobs = np.zeros((N_CORES, P, S), dtype=np.int32)
    nvecs = np.zeros((N_CORES, P, NT), dtype=f32)
    ids = np.zeros((N_CORES, NT, P), dtype=np.int64)
    off = 0
    for t in range(NT):
        base = t * P * N_CORES
        for core in range(N_CORES):
            sel = order[base + core + N_CORES * np.arange(P)]
            ids[core, t] = sel
            nvecs[core, :, t] = lens[sel]
            idx_blobs[core, :, off:off + Lb[t]] = idx_pad[sel, :Lb[t]]
        off += Lb[t]
    return table, idx_blobs, nvecs, Lb, S, ids


def _build_program(Lb, S):
    import concourse.bass as bass
    import concourse.bacc as bacc
    import concourse.mybir as mybir
    from concourse.tile import TileContext

    dt = mybir.dt
    Alu = mybir.AluOpType
    Act = mybir.ActivationFunctionType

    nc = bacc.Bacc("TRN2", target_bir_lowering=False, debug=False,
                   enable_asserts=False)
    table = nc.dram_tensor("table", [N + 1, 8], dt.float32, kind="ExternalInput")
    idx = nc.dram_tensor("idx", [P, S], dt.int32, kind="ExternalInput")
    nvec_d = nc.dram_tensor("nvec", [P, NT], dt.float32, kind="ExternalInput")
    res = nc.dram_tensor("res", [P, 19 * NT], dt.float32, kind="ExternalOutput")

    TINY = 1e-30

    with TileContext(nc) as tc:
        with tc.tile_pool(name="ret", bufs=1) as ret, \
             tc.tile_pool(name="gp", bufs=3) as gp, \
             tc.tile_pool(name="ip", bufs=3) as ip, \
             tc.tile_pool(name="sp", bufs=2) as sp:

            def nt_tile(tag, k=1):
                return ret.tile([P, k * NT], dt.float32, tag=tag, name=tag)

            NV = nt_tile("NV")      # n
            RN = nt_tile("RN")      # 1/n
            SUMS = nt_tile("SUMS", 4)   # sx, sy, sz, sv   ((f, t) layout)
            OH = nt_tile("OH", 4)       # counts sem==1..4
            PROD = nt_tile("PROD", 7)   # xx, xy, xz, yy, yz, zz, vv
            CEN = nt_tile("CEN", 3)     # cx, cy, cz
            SCRAW = nt_tile("SCRAW")

            nc.sync.dma_start(out=NV[:], in_=nvec_d[:, :])
            nc.vector.reciprocal(RN[:], NV[:])

            xcs = []
            off = 0
            for t in range(NT):
                lb = int(Lb[t])
                it = ip.tile([P, lb], dt.int32, tag="idx", name=f"it{t}")
                nc.sync.dma_start(out=it[:], in_=idx[:, off:off + lb])
                G = gp.tile([P, lb * 8], dt.float32, tag="G", name=f"G{t}")
                for l in range(lb):
                    nc.gpsimd.indirect_dma_start(
                        out=G[:, l * 8:(l + 1) * 8],
                        out_offset=None,
                        in_=table[:, :],
                        in_offset=bass.IndirectOffsetOnAxis(ap=it[:, l:l + 1], axis=0),
                    )
                Gf = G[:].rearrange("p (l f) -> p f l", f=8)  # [P, 8, lb]

                # raw sums of x,y,z,v and oh1..4  (reduce innermost = l)
                nc.vector.tensor_reduce(
                    out=SUMS[:].rearrange("p (f t) -> p f t", t=NT)[:, :, t],
                    in_=Gf[:, 0:4, :], axis=mybir.AxisListType.X, op=Alu.add)
                nc.vector.tensor_reduce(
                    out=OH[:].rearrange("p (f t) -> p f t", t=NT)[:, :, t],
                    in_=Gf[:, 4:8, :], axis=mybir.AxisListType.X, op=Alu.add)

                # products with fused accumulate
                scratch = sp.tile([P, lb], dt.float32, tag="scr", name=f"scr{t}")
                pairs = [(0, 0), (0, 1), (0, 2), (1, 1), (1, 2), (2, 2), (3, 3)]
                for q, (i, j) in enumerate(pairs):
                    nc.vector.scalar_tensor_tensor(
                        out=scratch[:],
                        in0=Gf[:, i, :], scalar=1.0, in1=Gf[:, j, :],
                        op0=Alu.mult, op1=Alu.mult,
                        accum_out=PROD[:, q * NT + t:q * NT + t + 1])

                # center = sums * (1/n); xc_i = x_i - center_i  (retained)
                nc.vector.tensor_scalar(
                    out=CEN[:].rearrange("p (f t) -> p f t", t=NT)[:, :, t],
                    in0=SUMS[:].rearrange("p (f t) -> p f t", t=NT)[:, 0:3, t],
                    scalar1=RN[:, t:t + 1], scalar2=None, op0=Alu.mult)
                xc = ret.tile([P, 3 * lb], dt.float32, tag=f"xc{t}", name=f"xc{t}")
                for i in range(3):
                    nc.vector.tensor_scalar(
                        out=xc[:, i * lb:(i + 1) * lb],
                        in0=Gf[:, i, :],
                        scalar1=CEN[:, i * NT + t:i * NT + t + 1],
                        scalar2=None, op0=Alu.subtract)
                xcs.append(xc)
                off += lb

            # ---------- batched per-cluster math on [P, NT] ----------
            def tmp(tag, k=1):
                return ret.tile([P, k * NT], dt.float32, tag=tag, name=tag)

            def tt(op, out, a, b):
                nc.vector.tensor_tensor(out=out, in0=a, in1=b, op=op)

            def ts(out, in0, s, op):
                nc.vector.tensor_scalar(out=out, in0=in0, scalar1=s,
                                        scalar2=None, op0=op)

            def stt(out, in0, s, op0, op1, in1, accum=None):
                nc.vector.scalar_tensor_tensor(out=out, in0=in0, scalar=s,
                                               in1=in1, op0=op0, op1=op1,
                                               accum_out=accum)

            def act(out, in_, func, bias=0.0, scale=1.0):
                nc.scalar.activation(out, in_, func, bias=bias, scale=scale)

            def sl(T, i):  # [P, NT] slice i of a k*NT tile
                return T[:, i * NT:(i + 1) * NT]

            A = nt_tile("A", 6)   # axx axy axz ayy ayz azz
            # a_ij = prod_ij - c_i * s_j
            cmap = [(0, 0, 0), (1, 0, 1), (2, 0, 2), (3, 1, 1), (4, 1, 2), (5, 2, 2)]
            SC1 = tmp("SC1")
            for q, i, j in cmap:
                tt(Alu.mult, SC1[:], sl(CEN, i), sl(SUMS, j))
                tt(Alu.subtract, sl(A, q), sl(PROD, q), SC1[:])

            # value stats
            MEANV = tmp("MEANV"); STDV = tmp("STDV")
            tt(Alu.mult, MEANV[:], sl(SUMS, 3), RN[:])
            VAR = tmp("VAR")
            tt(Alu.mult, VAR[:], MEANV[:], sl(SUMS, 3))
            tt(Alu.subtract, VAR[:], sl(PROD, 6), VAR[:])
            NM1 = tmp("NM1")
            ts(NM1[:], NV[:], 1.0, Alu.subtract)
            nc.vector.reciprocal(SC1[:], NM1[:])
            tt(Alu.mult, VAR[:], VAR[:], SC1[:])
            ts(VAR[:], VAR[:], 0.0, Alu.max)
            act(STDV[:], VAR[:], Act.Sqrt)

            # mode of semantic class scan tiles (filled later, after eig temps)
            MODE = tmp("MODE"); BEST = tmp("BEST"); GT = tmp("GT"); KT = tmp("KT")

            # eigenvalues: trig closed form
            Q = tmp("Q"); P1 = tmp("P1"); P2 = tmp("P2"); PP = tmp("PP")
            RP = tmp("RP"); DET = tmp("DET"); RR = tmp("RR"); SS = tmp("SS")
            AT = tmp("AT"); PHI = tmp("PHI")
            W0 = tmp("W0"); W1 = tmp("W1"); W2 = tmp("W2"); RW2 = tmp("RW2")
            DIRWT = tmp("DIRWT")
            B6 = nt_tile("B6", 6)
            NB = nt_tile("NB", 6)  # normalized (A - qI)/p entries

            tt(Alu.add, Q[:], sl(A, 0), sl(A, 3))
            tt(Alu.add, Q[:], Q[:], sl(A, 5))
            ts(Q[:], Q[:], 1.0 / 3.0, Alu.mult)

            tt(Alu.mult, P1[:], sl(A, 1), sl(A, 1))
            tt(Alu.mult, SC1[:], sl(A, 2), sl(A, 2))
            tt(Alu.add, P1[:], P1[:], SC1[:])
            tt(Alu.mult, SC1[:], sl(A, 4), sl(A, 4))
            tt(Alu.add, P1[:], P1[:], SC1[:])

            # bxx,byy,bzz = diag - q ; p2 = bxx^2+byy^2+bzz^2 + 2 p1
            BD = nt_tile("BD", 3)
            tt(Alu.subtract, sl(BD, 0), sl(A, 0), Q[:])
            tt(Alu.subtract, sl(BD, 1), sl(A, 3), Q[:])
            tt(Alu.subtract, sl(BD, 2), sl(A, 5), Q[:])
            tt(Alu.mult, P2[:], sl(BD, 0), sl(BD, 0))
            tt(Alu.mult, SC1[:], sl(BD, 1), sl(BD, 1))
            tt(Alu.add, P2[:], P2[:], SC1[:])
            tt(Alu.mult, SC1[:], sl(BD, 2), sl(BD, 2))
            tt(Alu.add, P2[:], P2[:], SC1[:])
            stt(P2[:], P1[:], 2.0, Alu.mult, Alu.add, P2[:])
            ts(PP[:], P2[:], 1.0 / 6.0, Alu.mult)
            act(PP[:], PP[:], Act.Sqrt)
            ts(SC1[:], PP[:], TINY, Alu.max)
            nc.vector.reciprocal(RP[:], SC1[:])

            # normalized matrix entries
            tt(Alu.mult, sl(NB, 0), sl(BD, 0), RP[:])
            tt(Alu.mult, sl(NB, 1), sl(A, 1), RP[:])
            tt(Alu.mult, sl(NB, 2), sl(A, 2), RP[:])
            tt(Alu.mult, sl(NB, 3), sl(BD, 1), RP[:])
            tt(Alu.mult, sl(NB, 4), sl(A, 4), RP[:])
            tt(Alu.mult, sl(NB, 5), sl(BD, 2), RP[:])

            # det(NB): nxx(nyy nzz - nyz^2) - nxy(nxy nzz - nyz nxz)
            #          + nxz(nxy nyz - nyy nxz)
            SC2 = tmp("SC2"); SC3 = tmp("SC3")
            tt(Alu.mult, SC1[:], sl(NB, 3), sl(NB, 5))
            tt(Alu.mult, SC2[:], sl(NB, 4), sl(NB, 4))
            tt(Alu.subtract, SC1[:], SC1[:], SC2[:])
            tt(Alu.mult, DET[:], sl(NB, 0), SC1[:])
            tt(Alu.mult, SC1[:], sl(NB, 1), sl(NB, 5))
            tt(Alu.mult, SC2[:], sl(NB, 4), sl(NB, 2))
            tt(Alu.subtract, SC1[:], SC1[:], SC2[:])
            tt(Alu.mult, SC1[:], sl(NB, 1), SC1[:])
            tt(Alu.subtract, DET[:], DET[:], SC1[:])
            tt(Alu.mult, SC1[:], sl(NB, 1), sl(NB, 4))
            tt(Alu.mult, SC2[:], sl(NB, 3), sl(NB, 2))
            tt(Alu.subtract, SC1[:], SC1[:], SC2[:])
            tt(Alu.mult, SC1[:], sl(NB, 2), SC1[:])
            tt(Alu.add, DET[:], DET[:], SC1[:])

            ts(RR[:], DET[:], 0.5, Alu.mult)
            ts(RR[:], RR[:], -1.0, Alu.max)
            ts(RR[:], RR[:], 1.0, Alu.min)
            # s = sqrt(max(1 - r^2, 0)) ; at = atan(r / max(s, tiny))
            tt(Alu.mult, SS[:], RR[:], RR[:])
            nc.vector.tensor_scalar(out=SS[:], in0=SS[:], scalar1=-1.0,
                                    scalar2=1.0, op0=Alu.mult, op1=Alu.add)
            ts(SS[:], SS[:], 0.0, Alu.max)
            act(SS[:], SS[:], Act.Sqrt)
            # AT = asin(r) via atan with range reduction (ACT atan domain is
            # [-pi/2, pi/2]): z = min(|r|/s, s/|r|) <= 1; atan(z); piece back.
            UA = tmp("UA"); UB = tmp("UB")
            ts(SC1[:], RR[:], -1.0, Alu.mult)
            tt(Alu.max, SC1[:], SC1[:], RR[:])          # |r|
            ts(SS[:], SS[:], TINY, Alu.max)
            nc.vector.reciprocal(SC2[:], SS[:])         # 1/s
            tt(Alu.mult, UA[:], SC1[:], SC2[:])         # |r|/s
            ts(SC1[:], UA[:], TINY, Alu.max)
            nc.vector.reciprocal(UB[:], SC1[:])         # ~ s/|r|
            tt(Alu.min, SC2[:], UA[:], UB[:])           # z in [0,1]
            act(SC2[:], SC2[:], Act.Arctan)             # a = atan(z)
            ts(SC1[:], UA[:], 1.0, Alu.is_gt)           # cond: |r| > s
            nc.vector.tensor_scalar(out=SC3[:], in0=SC2[:], scalar1=-2.0,
                                    scalar2=_PI / 2.0, op0=Alu.mult, op1=Alu.add)
            tt(Alu.mult, SC3[:], SC3[:], SC1[:])
            tt(Alu.add, SC2[:], SC2[:], SC3[:])         # asin(|r|)
            ts(SC3[:], RR[:], 0.0, Alu.is_lt)
            nc.vector.tensor_scalar(out=SC3[:], in0=SC3[:], scalar1=-2.0,
                                    scalar2=1.0, op0=Alu.mult, op1=Alu.add)
            tt(Alu.mult, AT[:], SC2[:], SC3[:])         # asin(r)
            # phi = (pi/2 - at)/3 ; w2 = q + 2 p sin(phi + pi/2)
            # fold the sin phase offsets into the fused multiply-add
            nc.vector.tensor_scalar(out=PHI[:], in0=AT[:], scalar1=-1.0 / 3.0,
                                    scalar2=_PI / 6.0 + _PI / 2.0,
                                    op0=Alu.mult, op1=Alu.add)
            act(SC1[:], PHI[:], Act.Sin)
            tt(Alu.mult, SC1[:], SC1[:], PP[:])
            stt(W2[:], SC1[:], 2.0, Alu.mult, Alu.add, Q[:])
            # w0-cos: sin(phi + pi/2 + 2pi/3) = -sin(phi + pi/6) (range fix)
            nc.vector.tensor_scalar(out=PHI[:], in0=AT[:], scalar1=-1.0 / 3.0,
                                    scalar2=_PI / 6.0 + _PI / 6.0,
                                    op0=Alu.mult, op1=Alu.add)
            act(SC1[:], PHI[:], Act.Sin)
            tt(Alu.mult, SC1[:], SC1[:], PP[:])
            stt(W0[:], SC1[:], -2.0, Alu.mult, Alu.add, Q[:])
            ts(SC1[:], Q[:], 3.0, Alu.mult)
            tt(Alu.subtract, W1[:], SC1[:], W0[:])
            tt(Alu.subtract, W1[:], W1[:], W2[:])

            ts(SC1[:], W2[:], TINY, Alu.max)
            nc.vector.reciprocal(RW2[:], SC1[:])
            tt(Alu.mult, DIRWT[:], W1[:], RW2[:])
            nc.vector.tensor_scalar(out=DIRWT[:], in0=DIRWT[:], scalar1=-1.0,
                                    scalar2=1.0, op0=Alu.mult, op1=Alu.add)
            for q in range(6):
                tt(Alu.mult, sl(B6, q), sl(A, q), RW2[:])

            # principal eigenvector: M = (A - w0 I)(A - w1 I); best column
            CD = nt_tile("CD", 3)  # diag(A) - w0
            DD = nt_tile("DD", 3)  # diag(A) - w1
            for qi, ai in enumerate((0, 3, 5)):
                tt(Alu.subtract, sl(CD, qi), sl(A, ai), W0[:])
                tt(Alu.subtract, sl(DD, qi), sl(A, ai), W1[:])
            M9 = nt_tile("M9", 9)  # columns of M (3 cols x 3 rows)
            # col0 = (A - w0) @ (dxx, axy, axz)
            def mcol(colq, dv):
                # column j of M: M[:, j] = C @ d where C = A - w0 I with
                # C rows: (cxx, axy, axz), (axy, cyy, ayz), (axz, ayz, czz)
                crow = [(sl(CD, 0), sl(A, 1), sl(A, 2)),
                        (sl(A, 1), sl(CD, 1), sl(A, 4)),
                        (sl(A, 2), sl(A, 4), sl(CD, 2))]
                for r in range(3):
                    a0, a1, a2 = crow[r]
                    tt(Alu.mult, SC1[:], a0, dv[0])
                    tt(Alu.mult, SC2[:], a1, dv[1])
                    tt(Alu.add, SC1[:], SC1[:], SC2[:])
                    tt(Alu.mult, SC2[:], a2, dv[2])
                    tt(Alu.add, sl(M9, colq * 3 + r), SC1[:], SC2[:])

            mcol(0, (sl(DD, 0), sl(A, 1), sl(A, 2)))
            mcol(1, (sl(A, 1), sl(DD, 1), sl(A, 4)))
            mcol(2, (sl(A, 2), sl(A, 4), sl(DD, 2)))

            # column norms, pick argmax
            CN = nt_tile("CN", 3)
            for j in range(3):
                tt(Alu.mult, sl(CN, j), sl(M9, j * 3), sl(M9, j * 3))
                tt(Alu.mult, SC1[:], sl(M9, j * 3 + 1), sl(M9, j * 3 + 1))
                tt(Alu.add, sl(CN, j), sl(CN, j), SC1[:])
                tt(Alu.mult, SC1[:], sl(M9, j * 3 + 2), sl(M9, j * 3 + 2))
                tt(Alu.add, sl(CN, j), sl(CN, j), SC1[:])
            V3 = nt_tile("V3", 3)
            NBEST = tmp("NBEST")
            for i in range(3):
                nc.vector.tensor_copy(out=sl(V3, i), in_=sl(M9, i))
            nc.vector.tensor_copy(out=NBEST[:], in_=sl(CN, 0))
            for j in (1, 2):
                tt(Alu.is_gt, GT[:], sl(CN, j), NBEST[:])
                for i in range(3):
                    # v_i += gt * (m_i - v_i)
                    tt(Alu.subtract, SC1[:], sl(M9, j * 3 + i), sl(V3, i))
                    tt(Alu.mult, SC1[:], SC1[:], GT[:])
                    tt(Alu.add, sl(V3, i), sl(V3, i), SC1[:])
                tt(Alu.max, NBEST[:], NBEST[:], sl(CN, j))
            ts(SC1[:], NBEST[:], 1e-37, Alu.max)
            act(SC2[:], SC1[:], Act.Sqrt)
            nc.vector.reciprocal(SC2[:], SC2[:])
            for i in range(3):
                tt(Alu.mult, sl(V3, i), sl(V3, i), SC2[:])

            # mode of semantic class (ties -> smallest), counts are exact ints
            # count0 = n - oh1 - oh2 - oh3 - oh4
            tt(Alu.subtract, BEST[:], NV[:], sl(OH, 0))
            for k in (1, 2, 3):
                tt(Alu.subtract, BEST[:], BEST[:], sl(OH, k))
            nc.vector.memset(MODE[:], 0.0)
            for k in range(1, 5):
                ck = sl(OH, k - 1)
                tt(Alu.is_gt, GT[:], ck, BEST[:])
                nc.vector.tensor_scalar(out=KT[:], in0=MODE[:], scalar1=-1.0,
                                        scalar2=float(k), op0=Alu.mult,
                                        op1=Alu.add)
                tt(Alu.mult, KT[:], KT[:], GT[:])
                tt(Alu.add, MODE[:], MODE[:], KT[:])
                tt(Alu.max, BEST[:], BEST[:], ck)

            # ---------- pass B: sc accumulation per tile ----------
            for t in range(NT):
                lb = int(Lb[t])
                xc = xcs[t]
                xcx = xc[:, 0:lb]; xcy = xc[:, lb:2 * lb]; xcz = xc[:, 2 * lb:3 * lb]
                T = sp.tile([P, lb], dt.float32, tag="T", name=f"T{t}")
                S2 = sp.tile([P, lb], dt.float32, tag="S2", name=f"S2_{t}")
                S2b = sp.tile([P, lb], dt.float32, tag="S2b", name=f"S2b{t}")
                R = sp.tile([P, lb], dt.float32, tag="R", name=f"R{t}")
                nc.vector.tensor_scalar(out=T[:], in0=xcx,
                                        scalar1=V3[:, 0 * NT + t:0 * NT + t + 1],
                                        scalar2=None, op0=Alu.mult)
                stt(T[:], xcy, V3[:, 1 * NT + t:1 * NT + t + 1],
                    Alu.mult, Alu.add, T[:])
                stt(T[:], xcz, V3[:, 2 * NT + t:2 * NT + t + 1],
                    Alu.mult, Alu.add, T[:])
                stt(S2[:], xcx, 1.0, Alu.mult, Alu.mult, xcx)
                stt(S2b[:], xcy, 1.0, Alu.mult, Alu.mult, xcy)
                tt(Alu.add, S2[:], S2[:], S2b[:])
                stt(S2b[:], xcz, 1.0, Alu.mult, Alu.mult, xcz)
                tt(Alu.add, S2[:], S2[:], S2b[:])
                stt(S2b[:], T[:], 1.0, Alu.mult, Alu.mult, T[:])
                tt(Alu.subtract, S2[:], S2[:], S2b[:])
                ts(S2[:], S2[:], 0.0, Alu.max)
                act(R[:], S2[:], Act.Sqrt)
                stt(S2b[:], T[:], 1.0, Alu.mult, Alu.mult, R[:],
                    accum=SCRAW[:, t:t + 1])

            # ---------- final: pad correction, sign, outputs ----------
            T0 = tmp("T0"); CC = tmp("CC"); R0 = tmp("R0"); SCV = tmp("SCV")
            FAC = tmp("FAC")
            tt(Alu.mult, T0[:], sl(CEN, 0), sl(V3, 0))
            tt(Alu.mult, SC1[:], sl(CEN, 1), sl(V3, 1))
            tt(Alu.add, T0[:], T0[:], SC1[:])
            tt(Alu.mult, SC1[:], sl(CEN, 2), sl(V3, 2))
            tt(Alu.add, T0[:], T0[:], SC1[:])
            ts(T0[:], T0[:], -1.0, Alu.mult)   # t0 = -(c . v)
            tt(Alu.mult, CC[:], sl(CEN, 0), sl(CEN, 0))
            tt(Alu.mult, SC1[:], sl(CEN, 1), sl(CEN, 1))
            tt(Alu.add, CC[:], CC[:], SC1[:])
            tt(Alu.mult, SC1[:], sl(CEN, 2), sl(CEN, 2))
            tt(Alu.add, CC[:], CC[:], SC1[:])
            tt(Alu.mult, SC1[:], T0[:], T0[:])
            tt(Alu.subtract, R0[:], CC[:], SC1[:])
            ts(R0[:], R0[:], 0.0, Alu.max)
            act(R0[:], R0[:], Act.Sqrt)
            # npad = Lb - n  (per-tile constant Lb baked via iota-style memset)
            NPAD = tmp("NPAD")
            for t in range(NT):
                nc.vector.tensor_scalar(
                    out=NPAD[:, t:t + 1], in0=NV[:, t:t + 1], scalar1=-1.0,
                    scalar2=float(int(Lb[t])), op0=Alu.mult, op1=Alu.add)
            tt(Alu.mult, SC1[:], T0[:], R0[:])
            tt(Alu.mult, SC1[:], SC1[:], NPAD[:])
            tt(Alu.subtract, SCV[:], SCRAW[:], SC1[:])
            # fac = dirwt * (1 - 2 * (sc < 0))
            ts(GT[:], SCV[:], 0.0, Alu.is_lt)
            nc.vector.tensor_scalar(out=GT[:], in0=GT[:], scalar1=-2.0,
                                    scalar2=1.0, op0=Alu.mult, op1=Alu.add)
            tt(Alu.mult, FAC[:], DIRWT[:], GT[:])
            for i in range(3):
                tt(Alu.mult, sl(V3, i), sl(V3, i), FAC[:])

            # outputs: 19 planes of [P, NT]
            planes = [sl(CEN, 0), sl(CEN, 1), sl(CEN, 2),
                      sl(B6, 0), sl(B6, 1), sl(B6, 2),
                      sl(B6, 1), sl(B6, 3), sl(B6, 4),
                      sl(B6, 2), sl(B6, 4), sl(B6, 5),
                      sl(V3, 0), sl(V3, 1), sl(V3, 2),
                      NV[:], MEANV[:], STDV[:], MODE[:]]
            for j, pl in enumerate(planes):
                nc.sync.dma_start(out=res[:, j * NT:(j + 1) * NT], in_=pl)

    nc.compile()
    return nc


_cache = {}


def kernel(data, clust_idx, clust_len):
    table, idx_blobs, nvecs, Lb, S, ids = _host_prep(data, clust_idx, clust_len)

    key = tuple(int(x) for x in Lb)
    if key not in _cache:
        _cache[key] = _build_program(Lb, S)
    nc = _cache[key]

    from concourse.bass_utils import run_bass_kernel_spmd
    in_maps = [{"table": table, "idx": idx_blobs[c], "nvec": nvecs[c]}
               for c in range(N_CORES)]
    global _last
    _last = (nc, in_maps)
    res = run_bass_kernel_spmd(nc, in_maps, list(range(N_CORES)))

    out = np.zeros((C, 19), dtype=f32)
    for core in range(N_CORES):
        r = res.results[core]["res"].reshape(P, 19, NT)
        for t in range(NT):
            out[ids[core, t]] = r[:, :, t]
    return out
